# revision 10
# baseline (speedup 1.0000x reference)
"""Single-head attention on 8 TRN2 NeuronCores, batch-parallel (1 batch elem/core).

reference (per batch b):
  qp = q[b] @ w_q; kp = k[b] @ w_k; vp = v[b] @ w_v        # [S,F]@[F,DK] -> [S,DK]
  scores = qp @ kp.T / sqrt(DK)                            # [S,S]
  out[b] = softmax(scores, axis=-1) @ vp                   # [S,DK]

Shapes: B=8, S=2048, F=1024, DK=128. f32 in/out, bf16 compute, f32 accumulate.

v6 structure (per core). The 25MB input read paces the kernel until ~76us
(16 DMA engines at line rate); the job is to hide ALL compute under it and
minimize the post-load drain:
  * interleaved q/k group pipeline: after q-group c and k-group g land,
    scoresT(t, c) tiles (kpT_t stationary, qpT chunk moving, 512-wide) and
    their Exp run immediately - no separate Q phase, so ACT/DVE/PE work
    spreads across the whole DMA window.
  * v groups load last (drain after the final load is just 8 PV matmuls +
    finishes, ~4us, vs ~12us if k were last).
  * PV: vp tile stationary, expT moving 512-wide, accumulated transposed
    (outT [DK,sq]) over all 16 k-tiles in psum via start/stop; sq chunks
    0,1 stream with the v groups, chunks 2,3 + per-tile PE back-transpose
    and ACT 1/denom scaling form the tail, finishes interleaved.
  * softmax denominator: DVE colsum (bf16, +0.1% err) of expT tiles, 16
    tiny PE matmuls (colsum_chunk.T @ ones) -> denom per sq partition,
    DVE reciprocal; all during the v-load window.
  * all transposes on the PE (LDWEIGHTS queue overlaps; measured 56ns/t).
    XBAR DMA transposes lose: they starve behind the saturated loads.
  * v xT psum->sbuf copies on ACT (idle in the v window); q/k copies and
    colsum on DVE.
"""
import numpy as np

B, S, F, DK = 8, 2048, 1024, 128
P = 128
N_CORES = 8
GT = 4                 # s-tiles per group
NG = S // (P * GT)     # 4 groups per input
NF = F // P            # 8 f-chunks
NT = S // P            # 16 s-tiles
W4 = GT * P            # 512
SOFTMAX_SCALE = 1.0 / float(np.sqrt(DK))

_COMPILED = {}


def _build():
    import concourse.bass as bass
    import concourse.mybir as mybir
    from concourse import bacc
    from concourse.tile import TileContext
    from concourse.masks import make_identity

    f32 = mybir.dt.float32
    bf16 = mybir.dt.bfloat16
    EXP = mybir.ActivationFunctionType.Exp
    CPY = mybir.ActivationFunctionType.Copy
    ADD = mybir.AluOpType.add

    nc = bacc.Bacc("TRN2", target_bir_lowering=False, debug=False,
                   num_devices=N_CORES)
    q_ext = nc.dram_tensor("q", [S, F], f32, kind="ExternalInput").ap()
    k_ext = nc.dram_tensor("k", [S, F], f32, kind="ExternalInput").ap()
    v_ext = nc.dram_tensor("v", [S, F], f32, kind="ExternalInput").ap()
    wq_ext = nc.dram_tensor("w_q", [F, DK], f32, kind="ExternalInput").ap()
    wk_ext = nc.dram_tensor("w_k", [F, DK], f32, kind="ExternalInput").ap()
    wv_ext = nc.dram_tensor("w_v", [F, DK], f32, kind="ExternalInput").ap()
    out_ext = nc.dram_tensor("out", [S, DK], f32, kind="ExternalOutput").ap()

    q_view = q_ext.rearrange("(n p) f -> p n f", p=P)
    k_view = k_ext.rearrange("(n p) f -> p n f", p=P)
    v_view = v_ext.rearrange("(n p) f -> p n f", p=P)
    out_view = out_ext.rearrange("(n p) d -> p n d", p=P)

    with TileContext(nc) as tc:
        with (
            tc.tile_pool(name="const", bufs=1) as const,
            tc.tile_pool(name="persist", bufs=1) as persist,
            tc.tile_pool(name="stage", bufs=4) as stage,
            tc.tile_pool(name="xtp", bufs=2) as xtp,
            tc.tile_pool(name="outp", bufs=4) as outp,
            tc.tile_pool(name="tp_ps", bufs=2, space="PSUM") as tp_ps,
            tc.tile_pool(name="pj_ps", bufs=2, space="PSUM") as pj_ps,
            tc.tile_pool(name="sc_ps", bufs=2, space="PSUM") as sc_ps,
            tc.tile_pool(name="ac_ps", bufs=1, space="PSUM") as ac_ps,
        ):
            def load_group(view, g, nm):
                x_nat = stage.tile([P, GT, F], bf16, tag="stage", name=nm)
                h = GT // 2
                for hh in range(2):
                    nc.gpsimd.dma_start(
                        out=x_nat[:, hh * h:(hh + 1) * h, :],
                        in_=view[:, GT * g + hh * h:GT * g + (hh + 1) * h, :])
                return x_nat

            def load_weight(ext, nm):
                w_sb = const.tile([P, NF, DK], bf16, tag=nm, name=nm)
                nc.gpsimd.dma_start(out=w_sb[:, :, :],
                                    in_=ext.rearrange("(c p) d -> p c d", p=P))
                return w_sb

            # first loads go ahead of everything on the DMA queue
            q_nat0 = load_group(q_view, 0, "q_nat")
            wq_sb = load_weight(wq_ext, "wq")

            ident = const.tile([P, P], bf16)
            make_identity(nc, ident)
            ones_col = const.tile([P, 1], bf16)
            nc.gpsimd.memset(ones_col[:, :], 1.0)

            qpT = persist.tile([P, S], bf16)           # [DK, sq]
            kpT = persist.tile([P, S], bf16)           # [DK, sk]
            vp1 = persist.tile([P, NT, DK], bf16)      # [sk, tile, DK]
            expT_all = persist.tile([P, NT, S], bf16)  # [sk, sk-tile, sq]
            colsum = persist.tile([P, S], bf16)        # [sk, sq] partial denom
            rinv_all = persist.tile([P, NT], f32)      # [sq, tile]
            outT_sb = persist.tile([P, 4, W4], bf16)   # [DK, sq-chunk, 512]

            def transpose_group(x_nat, nm, copy_eng):
                # [P, GT, F] bf16 (s on parts) -> [P, NF, GT*P] (f on parts)
                xT = xtp.tile([P, NF, W4], bf16, tag="xT", name=nm)
                for cc in range(NF // 2):
                    tp = tp_ps.tile([P, 2, W4], bf16, tag="tp", name="tp")
                    for ci in range(2):
                        c = 2 * cc + ci
                        for t in range(GT):
                            nc.tensor.transpose(
                                tp[:, ci, t * P:(t + 1) * P],
                                x_nat[:, t, c * P:(c + 1) * P],
                                ident[:, :])
                    copy_eng(xT[:, 2 * cc:2 * cc + 2, :], tp[:, :, :])
                return xT

            def dve_copy(dst, src):
                nc.vector.tensor_copy(dst, src)

            def act_copy(dst, src):
                nc.scalar.copy(dst, src)

            def proj_qk(xT, w_sb, dstT, g):
                pj = pj_ps.tile([P, W4], f32, tag="pj", name="pj")
                for c in range(NF):
                    nc.tensor.matmul(pj[:, :], w_sb[:, c, :], xT[:, c, :],
                                     start=(c == 0), stop=(c == NF - 1))
                nc.scalar.copy(dstT[:, W4 * g:W4 * (g + 1)], pj[:, :])

            def proj_v(xT, g):
                for tl in range(GT):
                    vps = pj_ps.tile([P, W4], f32, tag="pj", name="vps")
                    for c in range(NF):
                        nc.tensor.matmul(vps[:, 0:DK],
                                         xT[:, c, tl * P:(tl + 1) * P],
                                         wv_sb[:, c, :],
                                         start=(c == 0), stop=(c == NF - 1))
                    nc.vector.tensor_copy(vp1[:, GT * g + tl, :], vps[:, 0:DK])

            colsum_started = set()

            def scores_exp_1(t, c):
                # one [sk=128, sq=512] scoresT tile + exp + colsum piece
                sc = sc_ps.tile([P, W4], f32, tag="sc", name="sc")
                nc.tensor.matmul(sc[:, :],
                                 kpT[:, t * P:(t + 1) * P],
                                 qpT[:, W4 * c:W4 * (c + 1)],
                                 start=True, stop=True)
                nc.scalar.activation(
                    expT_all[:, t, W4 * c:W4 * (c + 1)],
                    sc[:, :], EXP, scale=SOFTMAX_SCALE)
                cs = colsum[:, W4 * c:W4 * (c + 1)]
                ex = expT_all[:, t, W4 * c:W4 * (c + 1)]
                if c not in colsum_started:
                    colsum_started.add(c)
                    nc.vector.tensor_copy(cs, ex)
                else:
                    nc.vector.tensor_tensor(cs, cs, ex, ADD)

            def pv_tile(acc, t, chunk, start, stop):
                nc.tensor.matmul(
                    acc[:, :], vp1[:, t, :],
                    expT_all[:, t, W4 * chunk:W4 * (chunk + 1)],
                    start=start, stop=stop, skip_group_check=True)

            def finish(j):
                tp = tp_ps.tile([P, 2, W4], bf16, tag="tp", name="ftp")
                nc.tensor.transpose(
                    tp[:, 0, 0:P],
                    outT_sb[:, j // GT, (j % GT) * P:(j % GT + 1) * P],
                    ident[:, :])
                out_t = outp.tile([P, DK], f32, tag="out", name="out_t")
                nc.scalar.activation(out_t[:, :], tp[:, 0, 0:P], CPY,
                                     scale=rinv_all[:, j:j + 1])
                nc.sync.dma_start(out=out_view[:, j, :], in_=out_t[:, :])

            # ---- interleaved pipeline over segments:
            # qk0, qk1, v0, qk2, v1, qk3, v2, denom, v3
            # v(gv) PV needs exp(t in gv, chunks 0,1) => emitted after qk(gv+1)
            wk_sb = wv_sb = None
            accs1 = [
                ac_ps.tile([P, W4], f32, tag=f"out{i}", name=f"out{i}")
                for i in range(2)
            ]

            def qk_segment(g):
                nonlocal wk_sb
                q_nat = q_nat0 if g == 0 else load_group(q_view, g, "q_nat")
                k_nat = load_group(k_view, g, "k_nat")
                if g == 0:
                    wk_sb = load_weight(wk_ext, "wk")
                xTq = transpose_group(q_nat, "qT", dve_copy)
                proj_qk(xTq, wq_sb, qpT, g)
                xTk = transpose_group(k_nat, "kT", dve_copy)
                proj_qk(xTk, wk_sb, kpT, g)
                # L-shaped frontier: all (t, c) pairs newly unlocked
                for t in range(GT * g, GT * (g + 1)):
                    for c in range(g + 1):
                        scores_exp_1(t, c)
                for t in range(0, GT * g):
                    scores_exp_1(t, g)

            def v_segment(gv):
                nonlocal wv_sb
                v_nat = load_group(v_view, gv, "v_nat")
                if gv == 0:
                    wv_sb = load_weight(wv_ext, "wv")
                xTv = transpose_group(v_nat, "vT", act_copy)
                proj_v(xTv, gv)
                for t in range(GT * gv, GT * (gv + 1)):
                    for i in range(2):
                        pv_tile(accs1[i], t, i,
                                start=(t == 0), stop=(t == NT - 1))

            def denom_segment():
                for j in range(NT):
                    dn = sc_ps.tile([P, W4], f32, tag="sc", name="dn")
                    nc.tensor.matmul(dn[:, 0:1],
                                     colsum[:, j * P:(j + 1) * P],
                                     ones_col[:, :], start=True, stop=True)
                    nc.vector.reciprocal(rinv_all[:, j:j + 1], dn[:, 0:1])

            qk_segment(0)
            qk_segment(1)
            v_segment(0)
            qk_segment(2)
            v_segment(1)
            qk_segment(3)
            v_segment(2)
            denom_segment()
            v_segment(3)

            # ---- tail: chunks 2,3 with finishes interleaved ----
            for i in range(2):
                nc.vector.tensor_copy(outT_sb[:, i, :], accs1[i][:, :])

            acc2 = ac_ps.tile([P, W4], f32, tag="out0", name="acc2")
            for t in range(NT):
                pv_tile(acc2, t, 2, start=(t == 0), stop=(t == NT - 1))
            for j in range(4):
                finish(j)
            nc.vector.tensor_copy(outT_sb[:, 2, :], acc2[:, :])
            acc3 = ac_ps.tile([P, W4], f32, tag="out1", name="acc3")
            for t in range(NT):
                pv_tile(acc3, t, 3, start=(t == 0), stop=(t == NT - 1))
            for j in range(4, 12):
                finish(j)
            nc.vector.tensor_copy(outT_sb[:, 3, :], acc3[:, :])
            for j in range(12, 16):
                finish(j)

    nc.compile()
    return nc


def get_nc():
    if "nc" not in _COMPILED:
        _COMPILED["nc"] = _build()
    return _COMPILED["nc"]


def kernel(q, k, v, w_q, w_k, w_v):
    from concourse.bass_utils import run_bass_kernel_spmd

    q = np.ascontiguousarray(np.asarray(q, dtype=np.float32))
    k = np.ascontiguousarray(np.asarray(k, dtype=np.float32))
    v = np.ascontiguousarray(np.asarray(v, dtype=np.float32))
    w_q = np.ascontiguousarray(np.asarray(w_q, dtype=np.float32))
    w_k = np.ascontiguousarray(np.asarray(w_k, dtype=np.float32))
    w_v = np.ascontiguousarray(np.asarray(w_v, dtype=np.float32))

    nc = get_nc()
    in_maps = [
        {"q": q[b], "k": k[b], "v": v[b], "w_q": w_q, "w_k": w_k, "w_v": w_v}
        for b in range(B)
    ]
    res = run_bass_kernel_spmd(nc, in_maps, core_ids=list(range(N_CORES)))
    out = np.stack([res.results[b]["out"] for b in range(B)], axis=0)
    return out.astype(np.float32)


# revision 12
# speedup vs baseline: 1.2495x; 1.2495x over previous
"""Single-head attention on 8 TRN2 NeuronCores, batch-parallel (1 batch elem/core).

reference (per batch b):
  qp = q[b] @ w_q; kp = k[b] @ w_k; vp = v[b] @ w_v        # [S,F]@[F,DK] -> [S,DK]
  scores = qp @ kp.T / sqrt(DK)                            # [S,S]
  out[b] = softmax(scores, axis=-1) @ vp                   # [S,DK]

Shapes: B=8, S=2048, F=1024, DK=128. f32 in/out, bf16 compute, f32 accumulate.

v7 structure (per core). The 25MB input read paces the kernel (~76us; 16
DMA engines at line rate). Everything else must hide under it:
  * one interleaved pipeline (q0k0, q1k1, v0, q2k2, v1, q3k3, v2, dn, v3):
    scoresT(t,c) tiles unlock as q-chunk c and k-group t//4 project.
  * scoresT+exp emission is ZIPPED into transpose batches / projection
    matmuls: each [sk,512] scores matmul stalls the PE ~460ns waiting for
    ACT to drain the psum bank (exp is 680ns vs 216ns matmul), so scores
    are spread one-per-slot between non-ACT-gated PE work instead of
    emitted in stretches.
  * stage pool 6-deep: load stream stays continuous even when the PE
    transposes (which free stage slots) trail by a couple of groups.
  * PV: vp tile stationary, expT moving 512-wide, accumulated transposed
    (outT [DK,sq]) over all 16 k-tiles via psum start/stop; chunks 0,1
    stream with the v groups; chunks 2,3 + PE back-transpose + ACT
    1/denom scale form the tail, finishes interleaved.
  * denominator: DVE colsum (bf16) of exp tiles, 16 tiny PE matmuls
    (colsum_chunk.T @ ones), DVE reciprocal - all inside the load window.
  * all transposes on the PE: XBAR DMA transposes starve behind the
    saturated load queues (measured v3: 188us vs 135us baseline).
"""
import numpy as np

B, S, F, DK = 8, 2048, 1024, 128
P = 128
N_CORES = 8
GT = 4                 # s-tiles per group
NG = S // (P * GT)     # 4 groups per input
NF = F // P            # 8 f-chunks
NT = S // P            # 16 s-tiles
W4 = GT * P            # 512
SOFTMAX_SCALE = 1.0 / float(np.sqrt(DK))

_COMPILED = {}


def _build():
    import concourse.bass as bass
    import concourse.mybir as mybir
    from concourse import bacc
    from concourse.tile import TileContext
    from concourse.masks import make_identity

    f32 = mybir.dt.float32
    bf16 = mybir.dt.bfloat16
    EXP = mybir.ActivationFunctionType.Exp
    CPY = mybir.ActivationFunctionType.Copy
    ADD = mybir.AluOpType.add

    nc = bacc.Bacc("TRN2", target_bir_lowering=False, debug=False,
                   num_devices=N_CORES)
    q_ext = nc.dram_tensor("q", [S, F], f32, kind="ExternalInput").ap()
    k_ext = nc.dram_tensor("k", [S, F], f32, kind="ExternalInput").ap()
    v_ext = nc.dram_tensor("v", [S, F], f32, kind="ExternalInput").ap()
    wq_ext = nc.dram_tensor("w_q", [F, DK], f32, kind="ExternalInput").ap()
    wk_ext = nc.dram_tensor("w_k", [F, DK], f32, kind="ExternalInput").ap()
    wv_ext = nc.dram_tensor("w_v", [F, DK], f32, kind="ExternalInput").ap()
    out_ext = nc.dram_tensor("out", [S, DK], f32, kind="ExternalOutput").ap()

    q_view = q_ext.rearrange("(n p) f -> p n f", p=P)
    k_view = k_ext.rearrange("(n p) f -> p n f", p=P)
    v_view = v_ext.rearrange("(n p) f -> p n f", p=P)
    out_view = out_ext.rearrange("(n p) d -> p n d", p=P)

    with TileContext(nc) as tc:
        with (
            tc.tile_pool(name="const", bufs=1) as const,
            tc.tile_pool(name="persist", bufs=1) as persist,
            tc.tile_pool(name="stage", bufs=6) as stage,
            tc.tile_pool(name="xtp", bufs=3) as xtp,
            tc.tile_pool(name="outp", bufs=4) as outp,
            tc.tile_pool(name="tp_ps", bufs=2, space="PSUM") as tp_ps,
            tc.tile_pool(name="pj_ps", bufs=2, space="PSUM") as pj_ps,
            tc.tile_pool(name="sc_ps", bufs=2, space="PSUM") as sc_ps,
            tc.tile_pool(name="ac_ps", bufs=1, space="PSUM") as ac_ps,
        ):
            def load_group(view, g, nm):
                x_nat = stage.tile([P, GT, F], bf16, tag="stage", name=nm)
                h = GT // 2
                for hh in range(2):
                    nc.gpsimd.dma_start(
                        out=x_nat[:, hh * h:(hh + 1) * h, :],
                        in_=view[:, GT * g + hh * h:GT * g + (hh + 1) * h, :])
                return x_nat

            def load_weight(ext, nm):
                w_sb = const.tile([P, NF, DK], bf16, tag=nm, name=nm)
                nc.gpsimd.dma_start(out=w_sb[:, :, :],
                                    in_=ext.rearrange("(c p) d -> p c d", p=P))
                return w_sb

            # first loads go ahead of everything on the DMA queue
            q_nat0 = load_group(q_view, 0, "q_nat")
            wq_sb = load_weight(wq_ext, "wq")

            ident = const.tile([P, P], bf16)
            make_identity(nc, ident)
            ones_col = const.tile([P, 1], bf16)
            nc.gpsimd.memset(ones_col[:, :], 1.0)

            qpT = persist.tile([P, S], bf16)           # [DK, sq]
            kpT = persist.tile([P, S], bf16)           # [DK, sk]
            vp1 = persist.tile([P, NT, DK], bf16)      # [sk, tile, DK]
            expT_all = persist.tile([P, NT, S], bf16)  # [sk, sk-tile, sq]
            colsum = persist.tile([P, S], bf16)        # [sk, sq] partial denom
            rinv_all = persist.tile([P, NT], f32)      # [sq, tile]
            outT_sb = persist.tile([P, 4, W4], bf16)   # [DK, sq-chunk, 512]

            # --- deferred scoresT+exp emission, zipped into other PE work ---
            sc_queue = []
            sc_emitted = set()
            colsum_started = set()

            def scores_exp_1(t, c):
                # one [sk=128, sq=512] scoresT tile + exp + colsum piece
                sc = sc_ps.tile([P, W4], f32, tag="sc", name="sc")
                nc.tensor.matmul(sc[:, :],
                                 kpT[:, t * P:(t + 1) * P],
                                 qpT[:, W4 * c:W4 * (c + 1)],
                                 start=True, stop=True,
                                 skip_group_check=True)
                nc.scalar.activation(
                    expT_all[:, t, W4 * c:W4 * (c + 1)],
                    sc[:, :], EXP, scale=SOFTMAX_SCALE)
                cs = colsum[:, W4 * c:W4 * (c + 1)]
                ex = expT_all[:, t, W4 * c:W4 * (c + 1)]
                if c not in colsum_started:
                    colsum_started.add(c)
                    nc.vector.tensor_copy(cs, ex)
                else:
                    nc.vector.tensor_tensor(cs, cs, ex, ADD)

            def drain_sc(n=1):
                for _ in range(n):
                    if not sc_queue:
                        return
                    t, c = sc_queue.pop(0)
                    sc_emitted.add((t, c))
                    scores_exp_1(t, c)

            def require_sc(t, c):
                while (t, c) not in sc_emitted:
                    if not sc_queue:
                        raise RuntimeError(f"scores ({t},{c}) never queued")
                    drain_sc(1)

            def transpose_group(x_nat, nm):
                # [P, GT, F] bf16 (s on parts) -> [P, NF, GT*P] (f on parts)
                xT = xtp.tile([P, NF, W4], bf16, tag="xT", name=nm)
                for cc in range(NF // 2):
                    tp = tp_ps.tile([P, 2, W4], bf16, tag="tp", name="tp")
                    for ci in range(2):
                        c = 2 * cc + ci
                        for t in range(GT):
                            nc.tensor.transpose(
                                tp[:, ci, t * P:(t + 1) * P],
                                x_nat[:, t, c * P:(c + 1) * P],
                                ident[:, :])
                        drain_sc(1)
                    nc.vector.tensor_copy(xT[:, 2 * cc:2 * cc + 2, :], tp[:, :, :])
                return xT

            def proj_qk(xT, w_sb, dstT, g):
                pj = pj_ps.tile([P, W4], f32, tag="pj", name="pj")
                for c in range(NF):
                    nc.tensor.matmul(pj[:, :], w_sb[:, c, :], xT[:, c, :],
                                     start=(c == 0), stop=(c == NF - 1),
                                     skip_group_check=True)
                    drain_sc(1)
                nc.scalar.copy(dstT[:, W4 * g:W4 * (g + 1)], pj[:, :])

            def proj_v(xT, g):
                for tl in range(GT):
                    vps = pj_ps.tile([P, W4], f32, tag="pj", name="vps")
                    for c in range(NF):
                        nc.tensor.matmul(vps[:, 0:DK],
                                         xT[:, c, tl * P:(tl + 1) * P],
                                         wv_sb[:, c, :],
                                         start=(c == 0), stop=(c == NF - 1),
                                         skip_group_check=True)
                    nc.vector.tensor_copy(vp1[:, GT * g + tl, :], vps[:, 0:DK])
                    drain_sc(1)

            def pv_tile(acc, t, chunk, start, stop):
                nc.tensor.matmul(
                    acc[:, :], vp1[:, t, :],
                    expT_all[:, t, W4 * chunk:W4 * (chunk + 1)],
                    start=start, stop=stop, skip_group_check=True)

            def finish(j):
                tp = tp_ps.tile([P, 2, W4], bf16, tag="tp", name="ftp")
                nc.tensor.transpose(
                    tp[:, 0, 0:P],
                    outT_sb[:, j // GT, (j % GT) * P:(j % GT + 1) * P],
                    ident[:, :])
                out_t = outp.tile([P, DK], f32, tag="out", name="out_t")
                nc.scalar.activation(out_t[:, :], tp[:, 0, 0:P], CPY,
                                     scale=rinv_all[:, j:j + 1])
                nc.sync.dma_start(out=out_view[:, j, :], in_=out_t[:, :])

            # ---- interleaved pipeline ----
            wk_sb = wv_sb = None
            q_done = k_done = 0
            accs1 = [
                ac_ps.tile([P, W4], f32, tag=f"out{i}", name=f"out{i}")
                for i in range(2)
            ]

            def qk_segment(g):
                nonlocal wk_sb, q_done, k_done
                q_nat = q_nat0 if g == 0 else load_group(q_view, g, "q_nat")
                k_nat = load_group(k_view, g, "k_nat")
                if g == 0:
                    wk_sb = load_weight(wk_ext, "wk")
                xTq = transpose_group(q_nat, "qT")
                proj_qk(xTq, wq_sb, qpT, g)
                q_done = g + 1
                for t in range(GT * k_done):
                    sc_queue.append((t, g))
                xTk = transpose_group(k_nat, "kT")
                proj_qk(xTk, wk_sb, kpT, g)
                k_done = g + 1
                for t in range(GT * g, GT * (g + 1)):
                    for c in range(q_done):
                        sc_queue.append((t, c))

            def v_segment(gv):
                nonlocal wv_sb
                v_nat = load_group(v_view, gv, "v_nat")
                if gv == 0:
                    wv_sb = load_weight(wv_ext, "wv")
                xTv = transpose_group(v_nat, "vT")
                proj_v(xTv, gv)
                for t in range(GT * gv, GT * (gv + 1)):
                    for i in range(2):
                        require_sc(t, i)
                        pv_tile(accs1[i], t, i,
                                start=(t == 0), stop=(t == NT - 1))
                        drain_sc(1)

            def denom_segment():
                for j in range(NT):
                    dn = sc_ps.tile([P, W4], f32, tag="sc", name="dn")
                    nc.tensor.matmul(dn[:, 0:1],
                                     colsum[:, j * P:(j + 1) * P],
                                     ones_col[:, :], start=True, stop=True,
                                     skip_group_check=True)
                    nc.vector.reciprocal(rinv_all[:, j:j + 1], dn[:, 0:1])

            qk_segment(0)
            qk_segment(1)
            v_segment(0)
            qk_segment(2)
            v_segment(1)
            qk_segment(3)
            v_segment(2)
            drain_sc(len(sc_queue))
            denom_segment()
            v_segment(3)

            # ---- tail: chunks 2,3 with finishes interleaved ----
            for i in range(2):
                nc.vector.tensor_copy(outT_sb[:, i, :], accs1[i][:, :])

            acc2 = ac_ps.tile([P, W4], f32, tag="out0", name="acc2")
            for t in range(NT):
                pv_tile(acc2, t, 2, start=(t == 0), stop=(t == NT - 1))
            for j in range(4):
                finish(j)
            nc.vector.tensor_copy(outT_sb[:, 2, :], acc2[:, :])
            acc3 = ac_ps.tile([P, W4], f32, tag="out1", name="acc3")
            for t in range(NT):
                pv_tile(acc3, t, 3, start=(t == 0), stop=(t == NT - 1))
            for j in range(4, 12):
                finish(j)
            nc.vector.tensor_copy(outT_sb[:, 3, :], acc3[:, :])
            for j in range(12, 16):
                finish(j)

    nc.compile()
    return nc


def get_nc():
    if "nc" not in _COMPILED:
        _COMPILED["nc"] = _build()
    return _COMPILED["nc"]


def kernel(q, k, v, w_q, w_k, w_v):
    from concourse.bass_utils import run_bass_kernel_spmd

    q = np.ascontiguousarray(np.asarray(q, dtype=np.float32))
    k = np.ascontiguousarray(np.asarray(k, dtype=np.float32))
    v = np.ascontiguousarray(np.asarray(v, dtype=np.float32))
    w_q = np.ascontiguousarray(np.asarray(w_q, dtype=np.float32))
    w_k = np.ascontiguousarray(np.asarray(w_k, dtype=np.float32))
    w_v = np.ascontiguousarray(np.asarray(w_v, dtype=np.float32))

    nc = get_nc()
    in_maps = [
        {"q": q[b], "k": k[b], "v": v[b], "w_q": w_q, "w_k": w_k, "w_v": w_v}
        for b in range(B)
    ]
    res = run_bass_kernel_spmd(nc, in_maps, core_ids=list(range(N_CORES)))
    out = np.stack([res.results[b]["out"] for b in range(B)], axis=0)
    return out.astype(np.float32)
